# revision 38
# baseline (speedup 1.0000x reference)
"""Multi-head attention (B=2, N=2048, C=1024, H=16, D=64) on 8 trn2 cores.

Sharding: core c -> (batch b = c//4, head-group g = c%4 covering 4 heads).
Tensor-parallel over heads: Wq/Wk/Wv split column-wise, Wo row-wise; the
4 partial outputs per batch are summed on the host (+ bias).

Device layout trick: the host feeds activations TRANSPOSED ([C, seq]) so
every matmul on chip has its contraction dim on partitions with no
on-chip transposes:
  QT/KT panels [f, seq]  (projection outputs, transposed orientation)
  VP panel     [seq, f]  (natural orientation, +ones column per head)
  S^T  = Kh @ QhT        [sk, sq]  (d=64 contraction, 2-head row-packed)
  P^T  = exp(S^T * scale)          (ScalarE, reads PSUM directly)
  O'^T = [Vh|1]T-style   [65, sq]  (row 64 = softmax denominator)
  Y^T  = Wo^T @ (O^T/den)[o, seq]

Schedule: the kernel is ACT(exp)-bound in its attention phase (16.8M
exps/core at 1 elem/cycle/lane ~= 110us + per-instruction overheads)
and PE/DMA-bound in its projection phase.  The input DMA stream is
ordered kT, qT, vT so S matmuls for query chunk 0 start right after the
qT stream lands (~26us); the V projection drips into PE slack under the
first exp burst while vT streams, and the first head-pair's PVs catch
up (one per iteration) inside the second head-pair's pipeline.  All
PSUM drains run on DVE/gpsimd so ACT does exp and nothing else.
Output is written fp16 (halves output DMA); partials summed on host.
"""

import os
import sys

import numpy as np

sys.path.insert(0, "/opt/trn_rl_repo")

import concourse.bacc as bacc
import concourse.bass as bass
import concourse.tile as tile
from concourse import mybir
from concourse.bass_utils import run_bass_kernel_spmd

F32 = mybir.dt.float32
F16 = mybir.dt.float16

B = 2
SEQ = 2048
C = 1024
NH = 4          # heads per core
D = 64
FH = NH * D     # 256: feature slice per core
SCALE = D ** -0.5

N_CORES = 8
CCN = C // 128      # 8 contraction chunks
SQN = SEQ // 512    # 4 query chunks
SKN = SEQ // 128    # 16 key chunks

LAST_RESULTS = None  # stash for test harness introspection


def build_kernel(tc, qT, kT, vT, wq, wk, wv, wo, yT):
    nc = tc.nc

    with (
        tc.tile_pool(name="weights", bufs=1) as wpool,
        tc.tile_pool(name="panels", bufs=1) as panels,
        tc.tile_pool(name="xin", bufs=16) as xpool,
        tc.tile_pool(name="ptile", bufs=18) as ppool,
        tc.tile_pool(name="otile", bufs=3) as opool,
        tc.tile_pool(name="ytile", bufs=4) as ypool,
        tc.tile_pool(name="small", bufs=4) as small,
    ):
        # ---- resident weights ----
        wq_sb = wpool.tile([128, CCN, FH], F16, name="wq_sb", tag="wq")
        wk_sb = wpool.tile([128, CCN, FH], F16, name="wk_sb", tag="wk")
        wv_sb = wpool.tile([128, CCN, FH], F16, name="wv_sb", tag="wv")
        wo_sb = wpool.tile([128, 2, C], F16, name="wo_sb", tag="wo")

        # ---- persistent activation panels ----
        qt_sb = panels.tile([128, 2, SEQ], F16, name="qt_sb", tag="qt")   # [p, fc, sq] = QT
        kt_sb = panels.tile([128, 2, SEQ], F16, name="kt_sb", tag="kt")   # [p, fc, sk] = KT
        vp_sb = panels.tile([128, SKN, NH, D + 1], F16, name="vp_sb", tag="vp")  # V' natural
        nc.vector.memset(vp_sb[:, :, :, D:D + 1], 1.0)

        # ---- input DMA stream, priority-ordered and split across BOTH
        # hardware DGE queues (sync + act) for ~2x aggregate bandwidth.
        # K first (every S needs the full K panel), then Q (S on sqc0 needs
        # the full qT contraction), then V (PV only starts ~12us into the
        # exp stream), weights just ahead of their consumers.  The act
        # queue only dispatches descriptors from the ACT sequencer, well
        # before the exp stream begins.
        engs = (nc.sync, nc.scalar)
        nc.sync.dma_start(out=wk_sb, in_=wk[:, :].rearrange("(n p) m -> p n m", p=128))
        nc.scalar.dma_start(out=wq_sb, in_=wq[:, :].rearrange("(n p) m -> p n m", p=128))
        kx = []
        for cc in range(CCN):
            xin = xpool.tile([128, SEQ], F16, name="xin", tag="xin")
            engs[cc % 2].dma_start(out=xin, in_=kT[cc * 128:(cc + 1) * 128, :])
            kx.append(xin)
        qx = []
        for cc in range(CCN):
            xin = xpool.tile([128, SEQ], F16, name="xin", tag="xin")
            engs[cc % 2].dma_start(out=xin, in_=qT[cc * 128:(cc + 1) * 128, :])
            qx.append(xin)
        nc.sync.dma_start(out=wv_sb, in_=wv[:, :].rearrange("(n p) m -> p n m", p=128))
        vx = []
        for cc in range(CCN):
            xin = xpool.tile([128, SEQ], F16, name="xin", tag="xin")
            engs[cc % 2].dma_start(out=xin, in_=vT[cc * 128:(cc + 1) * 128, :])
            vx.append(xin)
        nc.scalar.dma_start(out=wo_sb, in_=wo[:, :].rearrange("(n p) m -> p n m", p=128))

        # preload the exp ACT table (~2.7us) during the DMA dead time so
        # the first real exp doesn't pay for it
        dummy = small.tile([1, 1], F16, name="dummy", tag="dummy")
        nc.scalar.activation(
            out=dummy,
            in_=vp_sb[0:1, 0, 0, D:D + 1],
            func=mybir.ActivationFunctionType.Exp,
            scale=1.0,
        )

        # ---- warm-up: the HAM power governor starts the core at k=4/n=8
        # duty (half-rate DMA *and* PE) and only grants full duty after a
        # few us of sustained tensor activity.  Burn dummy matmuls on a
        # memset tile from t~0 so the input DMA stream runs at full rate.
        warm_sb = panels.tile([128, 512], F16, name="warm_sb", tag="warm")
        nc.vector.memset(warm_sb, 1.0)
        with tc.tile_pool(name="ps_warm", bufs=1, space="PSUM") as ps_warm:
            wtile = ps_warm.tile([128, 512], F32, name="warm", tag="warmps")
            for _ in range(12):
                nc.tensor.matmul(
                    out=wtile,
                    lhsT=warm_sb[:, 0:128],
                    rhs=warm_sb,
                    start=True,
                    stop=True,
                )

        # ---- K projection: cc-outer so compute is paced by the arriving
        # chunk stream; all 8 PSUM accumulators live across chunks.
        with tc.tile_pool(name="ps_proj", bufs=8, space="PSUM") as ps_proj:
            acc = {}
            for cc in range(CCN):
                for fc in range(2):
                    for sqc in range(SQN):
                        if cc == 0:
                            acc[(fc, sqc)] = ps_proj.tile(
                                [128, 512], F32, name="pacc", tag="pacc"
                            )
                        nc.tensor.matmul(
                            out=acc[(fc, sqc)],
                            lhsT=wk_sb[:, cc, fc * 128:(fc + 1) * 128],
                            rhs=kx[cc][:, sqc * 512:(sqc + 1) * 512],
                            start=(cc == 0),
                            stop=(cc == CCN - 1),
                        )
            for fc in range(2):
                for sqc in range(SQN):
                    nc.vector.tensor_copy(
                        out=kt_sb[:, fc, sqc * 512:(sqc + 1) * 512],
                        in_=acc[(fc, sqc)],
                    )

        # ---- Q projection pass 1 (fc0 only, 4 banks): head-pair 0's S
        # matmuls need only the fc0 half of the Q panel, and a 4-bank pass
        # lets the attention s-pool open after just these 4 drains (a full
        # 8-bank pass would gate the first S matmul on all 8).
        with tc.tile_pool(name="ps_q1", bufs=4, space="PSUM") as ps_q1:
            acc = {}
            for cc in range(CCN):
                for sqc in range(SQN):
                    if cc == 0:
                        acc[sqc] = ps_q1.tile([128, 512], F32, name="qacc",
                                              tag="qacc")
                    nc.tensor.matmul(
                        out=acc[sqc],
                        lhsT=wq_sb[:, cc, 0:128],
                        rhs=qx[cc][:, sqc * 512:(sqc + 1) * 512],
                        start=(cc == 0),
                        stop=(cc == CCN - 1),
                    )
            for sqc in range(SQN):
                nc.vector.tensor_copy(
                    out=qt_sb[:, 0, sqc * 512:(sqc + 1) * 512],
                    in_=acc[sqc],
                )

        # ---- attention + output projection ----
        # Steady state per (sqc, hp): iter k emits S(k) -> exp(k) -> PV(k-1)
        # with the previous query-chunk's output projection drip-fed in, so
        # the PE stays dense while the ACT exp stream is the critical path.
        # sqc0 is special: hp0 is an S+exp burst with V-projection units
        # dripped in while vT streams; hp0's PVs then catch up one-per-iter
        # inside hp1's pipeline once the V panel is ready.
        with tc.tile_pool(name="ps_s", bufs=2, space="PSUM") as ps_s:
            def s_exp(sqc, hp, skc):
                sq = slice(sqc * 512, (sqc + 1) * 512)
                sk = slice(skc * 128, (skc + 1) * 128)
                s_ps = ps_s.tile([128, 1024], F32, name="sacc", tag="sacc")
                for h2 in range(2):
                    rows = slice(h2 * 64, (h2 + 1) * 64)
                    nc.tensor.matmul(
                        out=s_ps[:, h2 * 512:(h2 + 1) * 512],
                        lhsT=kt_sb[rows, hp, sk],
                        rhs=qt_sb[rows, hp, sq],
                        start=True,
                        stop=True,
                    )
                p_sb = ppool.tile([128, 1024], F16, name="p", tag="p")
                nc.scalar.activation(
                    out=p_sb,
                    in_=s_ps[:, :],
                    func=mybir.ActivationFunctionType.Exp,
                    scale=SCALE,
                )
                return p_sb

            def pv(o_ps, hp, pk, p_sb):
                for h2 in range(2):
                    nc.tensor.matmul(
                        out=o_ps[h2],
                        lhsT=vp_sb[:, pk, hp * 2 + h2, :],
                        rhs=p_sb[:, h2 * 512:(h2 + 1) * 512],
                        start=(pk == 0),
                        stop=(pk == SKN - 1),
                    )

            def normalize(o_ps, ot_sb, hp, h2_first=0):
                # rows 0..63 = O^T, row 64 = sum(exp).  Copy out of PSUM
                # first (early bank release), move the denominator row to
                # partition 0, reciprocal, broadcast across partitions on
                # gpsimd, and divide on DVE.  h2_first=1 starts the h2=1
                # chain (which ends in a cross-partition DMA) first - used
                # on the final window where that chain is the critical path.
                for h2 in (h2_first, 1 - h2_first):
                    o_sb = small.tile([D + 1, 512], F32, name="osb", tag="osb")
                    nc.vector.tensor_copy(out=o_sb, in_=o_ps[h2])
                    den0 = small.tile([1, 512], F32, name="den0", tag="den0")
                    nc.sync.dma_start(out=den0, in_=o_sb[D:D + 1, :])
                    rec = small.tile([1, 512], F32, name="rec", tag="rec")
                    nc.vector.reciprocal_approx_fast(out=rec, in_=den0)
                    rec_b = small.tile([D, 512], F32, name="recb", tag="recb")
                    nc.gpsimd.partition_broadcast(rec_b, rec)
                    if h2 == 0:
                        nc.vector.tensor_mul(
                            out=ot_sb[0:D, hp, :],
                            in0=o_sb[0:D, :],
                            in1=rec_b,
                        )
                    else:
                        tmp = small.tile([D, 512], F16, name="otmp", tag="otmp")
                        nc.vector.tensor_mul(out=tmp, in0=o_sb[0:D, :], in1=rec_b)
                        # cross-partition move (DVE lanes can't shift)
                        nc.sync.dma_start(out=ot_sb[D:128, hp, :], in_=tmp)

            def yproj_steps(ot_tile, sq_slice, oc):
                state = {}

                def mm0():
                    state["y_ps"] = ps_y.tile([128, 512], F32, name="yacc",
                                              tag="yacc")
                    nc.tensor.matmul(
                        out=state["y_ps"],
                        lhsT=wo_sb[:, 0, oc * 128:(oc + 1) * 128],
                        rhs=ot_tile[:, 0, :],
                        start=True,
                        stop=False,
                    )

                def mm1():
                    nc.tensor.matmul(
                        out=state["y_ps"],
                        lhsT=wo_sb[:, 1, oc * 128:(oc + 1) * 128],
                        rhs=ot_tile[:, 1, :],
                        start=False,
                        stop=True,
                    )
                    y_sb = ypool.tile([128, 512], F16, name="y", tag="y")
                    nc.vector.tensor_copy(out=y_sb, in_=state["y_ps"])
                    nc.sync.dma_start(
                        out=yT[oc * 128:(oc + 1) * 128, sq_slice], in_=y_sb
                    )

                return [mm0, mm1]

            # ---- sqc0, hp0: S+exp stream with Q pass 2 (fc1) and then the
            # V projection interleaved as PE filler.  The S stream is
            # ACT(exp)-paced via the s-pool; the dense Q2/V units slot
            # into the PE's idle time.  The two filler pools open
            # sequentially so PSUM stays within 8 banks (s4+q2_4, s4+v2).
            ot0_sb = opool.tile([128, 2, 512], F16, name="ot", tag="ot")
            p0 = {}
            with tc.tile_pool(name="ps_q2", bufs=4, space="PSUM") as ps_q2:
                q2acc = {}
                for skc in range(SKN):
                    p0[skc] = s_exp(0, 0, skc)
                    if skc < SKN - 4:
                        continue
                    for cc in (2 * (skc - (SKN - 4)),
                               2 * (skc - (SKN - 4)) + 1):
                        for sqc in range(SQN):
                            if cc == 0:
                                q2acc[sqc] = ps_q2.tile([128, 512], F32,
                                                        name="qacc", tag="qacc")
                            nc.tensor.matmul(
                                out=q2acc[sqc],
                                lhsT=wq_sb[:, cc, 128:256],
                                rhs=qx[cc][:, sqc * 512:(sqc + 1) * 512],
                                start=(cc == 0),
                                stop=(cc == CCN - 1),
                            )
                        if cc == CCN - 1:
                            for sqc in range(SQN):
                                nc.vector.tensor_copy(
                                    out=qt_sb[:, 1, sqc * 512:(sqc + 1) * 512],
                                    in_=q2acc[sqc],
                                )
            # V projection as a dense batch AFTER the whole S(0h0) crawl:
            # each V unit's last matmul stalls on the vT stream, and any S
            # emitted behind a stalled V in the in-order PE queue would
            # starve the exp feed.  After the crawl the PE is idle anyway
            # (exps self-pace), so the batch fills that window.
            with tc.tile_pool(name="ps_v", bufs=4, space="PSUM") as ps_v:
                for skc in range(SKN):
                    vacc = ps_v.tile([128, 256], F32, name="vacc", tag="vacc")
                    for cc in range(CCN):
                        nc.tensor.matmul(
                            out=vacc,
                            lhsT=vx[cc][:, skc * 128:(skc + 1) * 128],
                            rhs=wv_sb[:, cc, :],
                            start=(cc == 0),
                            stop=(cc == CCN - 1),
                        )
                    nc.vector.tensor_copy(
                        out=vp_sb[:, skc, :, 0:D],
                        in_=vacc.rearrange("p (h d) -> p h d", h=NH),
                    )

            # ---- sqc0, hp1: steady-interleaved, with hp0's PV catch-up
            # dripped one-per-iter; needs 4 o banks (both hps accumulate).
            with tc.tile_pool(name="ps_o4", bufs=4, space="PSUM") as ps_o4:
                o_ps0 = [
                    ps_o4.tile([D + 1, 512], F32, name="oacc", tag="oacc")
                    for _ in range(2)
                ]
                o_ps1 = [
                    ps_o4.tile([D + 1, 512], F32, name="oacc", tag="oacc")
                    for _ in range(2)
                ]
                p_prev = None
                for skc in range(SKN + 1):
                    if skc < SKN:
                        p_sb = s_exp(0, 1, skc)
                        pv(o_ps0, 0, skc, p0.pop(skc))
                    if skc >= 1:
                        pv(o_ps1, 1, skc - 1, p_prev)
                    if skc < SKN:
                        p_prev = p_sb
                normalize(o_ps0, ot0_sb, 0)
                normalize(o_ps1, ot0_sb, 1)

            # ---- sqc 1..3 steady state + dripped Y of previous sqc ----
            # The final sqc's Y is computed as two independent half-
            # contractions: the fc0 halves (which only need hp0's ot rows)
            # run start&stop with their partials drained to SBUF during
            # the hp1 exp stream; the tail is then just 8 fc1 matmuls with
            # a DVE add of the stashed partial during each drain.
            y0_sb = panels.tile([128, 8, 512], F32, name="y0_sb", tag="y0")
            with (
                tc.tile_pool(name="ps_o", bufs=2, space="PSUM") as ps_o,
                tc.tile_pool(name="ps_y", bufs=2, space="PSUM") as ps_y,
            ):
                def y_half0(ot_tile, oc):
                    def step():
                        y_ps = ps_y.tile([128, 512], F32, name="yacc",
                                         tag="yacc")
                        nc.tensor.matmul(
                            out=y_ps,
                            lhsT=wo_sb[:, 0, oc * 128:(oc + 1) * 128],
                            rhs=ot_tile[:, 0, :],
                            start=True,
                            stop=True,
                        )
                        nc.vector.tensor_copy(out=y0_sb[:, oc, :], in_=y_ps)
                    return step

                def y_half1(ot_tile, sq_slice, oc):
                    def step():
                        y_ps = ps_y.tile([128, 512], F32, name="yacc",
                                         tag="yacc")
                        nc.tensor.matmul(
                            out=y_ps,
                            lhsT=wo_sb[:, 1, oc * 128:(oc + 1) * 128],
                            rhs=ot_tile[:, 1, :],
                            start=True,
                            stop=True,
                        )
                        y_sb = ypool.tile([128, 512], F16, name="y", tag="y")
                        nc.vector.tensor_add(
                            out=y_sb, in0=y_ps, in1=y0_sb[:, oc, :]
                        )
                        nc.sync.dma_start(
                            out=yT[oc * 128:(oc + 1) * 128, sq_slice], in_=y_sb
                        )
                    return step

                prev_ot, prev_sq = ot0_sb, slice(0, 512)
                tail_units = []
                for sqc in range(1, SQN):
                    sq = slice(sqc * 512, (sqc + 1) * 512)
                    last_sqc = sqc == SQN - 1
                    ot_sb = opool.tile([128, 2, 512], F16, name="ot", tag="ot")
                    pending = [step for oc in range(8)
                               for step in yproj_steps(prev_ot, prev_sq, oc)]
                    for hp in range(2):
                        o_ps = [
                            ps_o.tile([D + 1, 512], F32, name="oacc", tag="oacc")
                            for _ in range(2)
                        ]
                        p_prev = None
                        every_iter = last_sqc and hp == 1
                        for skc in range(SKN + 1):
                            if skc < SKN:
                                p_sb = s_exp(sqc, hp, skc)
                            if skc >= 1:
                                pv(o_ps, hp, skc - 1, p_prev)
                            if skc < SKN:
                                p_prev = p_sb
                            if pending and skc >= 1 and (
                                every_iter or (skc + hp) % 2 == 1
                            ):
                                pending.pop(0)()
                        normalize(o_ps, ot_sb, hp,
                                  h2_first=1 if last_sqc else 0)
                        if last_sqc and hp == 0:
                            pending = pending + [y_half0(ot_sb, oc)
                                                 for oc in range(8)]
                    if last_sqc:
                        tail_units = [y_half1(ot_sb, sq, oc)
                                      for oc in range(8)]
                    for step in pending:
                        step()
                    prev_ot, prev_sq = ot_sb, sq
                # tail warm-up: the final normalize chain (DVE/gpsimd/DMA)
                # leaves the PE idle, which drops HAM back to half duty
                # right when the last Y matmuls run.  Dummy matmuls keep
                # the clock up; sized to roughly cover the chain latency
                # so the Y fc1 matmuls start right as their ot rows land.
                for _ in range(28):
                    wtile = ps_y.tile([128, 512], F32, name="warm",
                                      tag="yacc")
                    nc.tensor.matmul(
                        out=wtile,
                        lhsT=wo_sb[:, 0, 0:128],
                        rhs=wo_sb[:, 1, 0:512],
                        start=True,
                        stop=True,
                    )
                for step in tail_units:
                    step()


def build_bass():
    nc = bacc.Bacc("TRN2", target_bir_lowering=False, debug=False,
                   enable_asserts=False)
    qT = nc.dram_tensor("qT", [C, SEQ], F16, kind="ExternalInput").ap()
    kT = nc.dram_tensor("kT", [C, SEQ], F16, kind="ExternalInput").ap()
    vT = nc.dram_tensor("vT", [C, SEQ], F16, kind="ExternalInput").ap()
    wq = nc.dram_tensor("wq", [C, FH], F16, kind="ExternalInput").ap()
    wk = nc.dram_tensor("wk", [C, FH], F16, kind="ExternalInput").ap()
    wv = nc.dram_tensor("wv", [C, FH], F16, kind="ExternalInput").ap()
    wo = nc.dram_tensor("wo", [FH, C], F16, kind="ExternalInput").ap()
    yT = nc.dram_tensor("yT", [C, SEQ], F16, kind="ExternalOutput").ap()
    with tile.TileContext(nc) as tc:
        build_kernel(tc, qT, kT, vT, wq, wk, wv, wo, yT)
    nc.compile()
    return nc


_NC = None


def _get_nc():
    global _NC
    if _NC is None:
        _NC = build_bass()
    return _NC


def make_in_maps(q, k, v, Wq, Wk, Wv, Wo):
    f16 = np.float16
    in_maps = []
    for c in range(N_CORES):
        b, g = divmod(c, 4)
        fs = slice(g * FH, (g + 1) * FH)
        in_maps.append(dict(
            qT=np.ascontiguousarray(q[b].T).astype(f16),
            kT=np.ascontiguousarray(k[b].T).astype(f16),
            vT=np.ascontiguousarray(v[b].T).astype(f16),
            wq=np.ascontiguousarray(Wq[:, fs]).astype(f16),
            wk=np.ascontiguousarray(Wk[:, fs]).astype(f16),
            wv=np.ascontiguousarray(Wv[:, fs]).astype(f16),
            wo=np.ascontiguousarray(Wo[fs, :]).astype(f16),
        ))
    return in_maps


def kernel(q, k, v, Wq, Wk, Wv, Wo, bo):
    global LAST_RESULTS
    q = np.asarray(q, dtype=np.float32)
    k = np.asarray(k, dtype=np.float32)
    v = np.asarray(v, dtype=np.float32)
    Wq = np.asarray(Wq, dtype=np.float32)
    Wk = np.asarray(Wk, dtype=np.float32)
    Wv = np.asarray(Wv, dtype=np.float32)
    Wo = np.asarray(Wo, dtype=np.float32)
    bo = np.asarray(bo, dtype=np.float32)

    nc = _get_nc()
    in_maps = make_in_maps(q, k, v, Wq, Wk, Wv, Wo)
    res = run_bass_kernel_spmd(
        nc, in_maps, list(range(N_CORES)),
        trace=bool(os.environ.get("KERNEL_TRACE")),
    )
    LAST_RESULTS = res

    out = np.zeros((B, SEQ, C), dtype=np.float32)
    for c in range(N_CORES):
        out[c // 4] += res.results[c]["yT"].T.astype(np.float32)
    out += bo
    return out.astype(np.float32)


# revision 40
# speedup vs baseline: 1.0090x; 1.0090x over previous
"""Multi-head attention (B=2, N=2048, C=1024, H=16, D=64) on 8 trn2 cores.

Sharding: core c -> (batch b = c//4, head-group g = c%4 covering 4 heads).
Tensor-parallel over heads: Wq/Wk/Wv split column-wise, Wo row-wise; the
4 partial outputs per batch are summed on the host (+ bias).

Device layout trick: the host feeds activations TRANSPOSED ([C, seq]) so
every matmul on chip has its contraction dim on partitions with no
on-chip transposes:
  QT/KT panels [f, seq]  (projection outputs, transposed orientation)
  VP panel     [seq, f]  (natural orientation, +ones column per head)
  S^T  = Kh @ QhT        [sk, sq]  (d=64 contraction, 2-head row-packed)
  P^T  = exp(S^T * scale)          (ScalarE, reads PSUM directly)
  O'^T = [Vh|1]T-style   [65, sq]  (row 64 = softmax denominator)
  Y^T  = Wo^T @ (O^T/den)[o, seq]

Schedule: the kernel is ACT(exp)-bound in its attention phase (16.8M
exps/core at 1 elem/cycle/lane ~= 110us + per-instruction overheads)
and PE/DMA-bound in its projection phase.  The input DMA stream is
ordered kT, qT, vT so S matmuls for query chunk 0 start right after the
qT stream lands (~26us); the V projection drips into PE slack under the
first exp burst while vT streams, and the first head-pair's PVs catch
up (one per iteration) inside the second head-pair's pipeline.  All
PSUM drains run on DVE/gpsimd so ACT does exp and nothing else.
Output is written fp16 (halves output DMA); partials summed on host.
"""

import os
import sys

import numpy as np

sys.path.insert(0, "/opt/trn_rl_repo")

import concourse.bacc as bacc
import concourse.bass as bass
import concourse.tile as tile
from concourse import mybir
from concourse.bass_utils import run_bass_kernel_spmd

F32 = mybir.dt.float32
F16 = mybir.dt.float16

B = 2
SEQ = 2048
C = 1024
NH = 4          # heads per core
D = 64
FH = NH * D     # 256: feature slice per core
SCALE = D ** -0.5

N_CORES = 8
CCN = C // 128      # 8 contraction chunks
SQN = SEQ // 512    # 4 query chunks
SKN = SEQ // 128    # 16 key chunks

LAST_RESULTS = None  # stash for test harness introspection


def build_kernel(tc, qT, kT, vT, wq, wk, wv, wo, yT):
    nc = tc.nc

    with (
        tc.tile_pool(name="weights", bufs=1) as wpool,
        tc.tile_pool(name="panels", bufs=1) as panels,
        tc.tile_pool(name="xin", bufs=16) as xpool,
        tc.tile_pool(name="ptile", bufs=18) as ppool,
        tc.tile_pool(name="otile", bufs=3) as opool,
        tc.tile_pool(name="ytile", bufs=4) as ypool,
        tc.tile_pool(name="small", bufs=4) as small,
    ):
        # ---- resident weights ----
        wq_sb = wpool.tile([128, CCN, FH], F16, name="wq_sb", tag="wq")
        wk_sb = wpool.tile([128, CCN, FH], F16, name="wk_sb", tag="wk")
        wv_sb = wpool.tile([128, CCN, FH], F16, name="wv_sb", tag="wv")
        wo_sb = wpool.tile([128, 2, C], F16, name="wo_sb", tag="wo")

        # ---- persistent activation panels ----
        qt_sb = panels.tile([128, 2, SEQ], F16, name="qt_sb", tag="qt")   # [p, fc, sq] = QT
        kt_sb = panels.tile([128, 2, SEQ], F16, name="kt_sb", tag="kt")   # [p, fc, sk] = KT
        vp_sb = panels.tile([128, SKN, NH, D + 1], F16, name="vp_sb", tag="vp")  # V' natural
        nc.vector.memset(vp_sb[:, :, :, D:D + 1], 1.0)

        # ---- input DMA stream, priority-ordered and split across BOTH
        # hardware DGE queues (sync + act) for ~2x aggregate bandwidth.
        # K first (every S needs the full K panel), then Q (S on sqc0 needs
        # the full qT contraction), then V (PV only starts ~12us into the
        # exp stream), weights just ahead of their consumers.  The act
        # queue only dispatches descriptors from the ACT sequencer, well
        # before the exp stream begins.
        engs = (nc.sync, nc.scalar)
        nc.sync.dma_start(out=wk_sb, in_=wk[:, :].rearrange("(n p) m -> p n m", p=128))
        nc.scalar.dma_start(out=wq_sb, in_=wq[:, :].rearrange("(n p) m -> p n m", p=128))
        kx = []
        for cc in range(CCN):
            xin = xpool.tile([128, SEQ], F16, name="xin", tag="xin")
            engs[cc % 2].dma_start(out=xin, in_=kT[cc * 128:(cc + 1) * 128, :])
            kx.append(xin)
        qx = []
        for cc in range(CCN):
            xin = xpool.tile([128, SEQ], F16, name="xin", tag="xin")
            engs[cc % 2].dma_start(out=xin, in_=qT[cc * 128:(cc + 1) * 128, :])
            qx.append(xin)
        nc.sync.dma_start(out=wv_sb, in_=wv[:, :].rearrange("(n p) m -> p n m", p=128))
        vx = []
        for cc in range(CCN):
            xin = xpool.tile([128, SEQ], F16, name="xin", tag="xin")
            engs[cc % 2].dma_start(out=xin, in_=vT[cc * 128:(cc + 1) * 128, :])
            vx.append(xin)
        nc.scalar.dma_start(out=wo_sb, in_=wo[:, :].rearrange("(n p) m -> p n m", p=128))

        # preload the exp ACT table (~2.7us) during the DMA dead time so
        # the first real exp doesn't pay for it
        dummy = small.tile([1, 1], F16, name="dummy", tag="dummy")
        nc.scalar.activation(
            out=dummy,
            in_=vp_sb[0:1, 0, 0, D:D + 1],
            func=mybir.ActivationFunctionType.Exp,
            scale=1.0,
        )

        # ---- warm-up: the HAM power governor starts the core at k=4/n=8
        # duty (half-rate DMA *and* PE) and only grants full duty after a
        # few us of sustained tensor activity.  Burn dummy matmuls on a
        # memset tile from t~0 so the input DMA stream runs at full rate.
        warm_sb = panels.tile([128, 512], F16, name="warm_sb", tag="warm")
        nc.vector.memset(warm_sb, 1.0)
        with tc.tile_pool(name="ps_warm", bufs=1, space="PSUM") as ps_warm:
            wtile = ps_warm.tile([128, 512], F32, name="warm", tag="warmps")
            for _ in range(12):
                nc.tensor.matmul(
                    out=wtile,
                    lhsT=warm_sb[:, 0:128],
                    rhs=warm_sb,
                    start=True,
                    stop=True,
                )

        # ---- K projection: cc-outer so compute is paced by the arriving
        # chunk stream; all 8 PSUM accumulators live across chunks.
        with tc.tile_pool(name="ps_proj", bufs=8, space="PSUM") as ps_proj:
            acc = {}
            for cc in range(CCN):
                for fc in range(2):
                    for sqc in range(SQN):
                        if cc == 0:
                            acc[(fc, sqc)] = ps_proj.tile(
                                [128, 512], F32, name="pacc", tag="pacc"
                            )
                        nc.tensor.matmul(
                            out=acc[(fc, sqc)],
                            lhsT=wk_sb[:, cc, fc * 128:(fc + 1) * 128],
                            rhs=kx[cc][:, sqc * 512:(sqc + 1) * 512],
                            start=(cc == 0),
                            stop=(cc == CCN - 1),
                        )
            for fc in range(2):
                for sqc in range(SQN):
                    nc.vector.tensor_copy(
                        out=kt_sb[:, fc, sqc * 512:(sqc + 1) * 512],
                        in_=acc[(fc, sqc)],
                    )

        # ---- Q projection pass 1 (fc0 only, 4 banks): head-pair 0's S
        # matmuls need only the fc0 half of the Q panel, and a 4-bank pass
        # lets the attention s-pool open after just these 4 drains (a full
        # 8-bank pass would gate the first S matmul on all 8).
        with tc.tile_pool(name="ps_q1", bufs=4, space="PSUM") as ps_q1:
            acc = {}
            for cc in range(CCN):
                for sqc in range(SQN):
                    if cc == 0:
                        acc[sqc] = ps_q1.tile([128, 512], F32, name="qacc",
                                              tag="qacc")
                    nc.tensor.matmul(
                        out=acc[sqc],
                        lhsT=wq_sb[:, cc, 0:128],
                        rhs=qx[cc][:, sqc * 512:(sqc + 1) * 512],
                        start=(cc == 0),
                        stop=(cc == CCN - 1),
                    )
            for sqc in range(SQN):
                nc.vector.tensor_copy(
                    out=qt_sb[:, 0, sqc * 512:(sqc + 1) * 512],
                    in_=acc[sqc],
                )

        # ---- attention + output projection ----
        # Steady state per (sqc, hp): iter k emits S(k) -> exp(k) -> PV(k-1)
        # with the previous query-chunk's output projection drip-fed in, so
        # the PE stays dense while the ACT exp stream is the critical path.
        # sqc0 is special: hp0 is an S+exp burst with V-projection units
        # dripped in while vT streams; hp0's PVs then catch up one-per-iter
        # inside hp1's pipeline once the V panel is ready.
        with tc.tile_pool(name="ps_s", bufs=2, space="PSUM") as ps_s:
            def s_exp(sqc, hp, skc):
                sq = slice(sqc * 512, (sqc + 1) * 512)
                sk = slice(skc * 128, (skc + 1) * 128)
                s_ps = ps_s.tile([128, 1024], F32, name="sacc", tag="sacc")
                for h2 in range(2):
                    rows = slice(h2 * 64, (h2 + 1) * 64)
                    nc.tensor.matmul(
                        out=s_ps[:, h2 * 512:(h2 + 1) * 512],
                        lhsT=kt_sb[rows, hp, sk],
                        rhs=qt_sb[rows, hp, sq],
                        start=True,
                        stop=True,
                    )
                p_sb = ppool.tile([128, 1024], F16, name="p", tag="p")
                nc.scalar.activation(
                    out=p_sb,
                    in_=s_ps[:, :],
                    func=mybir.ActivationFunctionType.Exp,
                    scale=SCALE,
                )
                return p_sb

            def pv(o_ps, hp, pk, p_sb):
                for h2 in range(2):
                    nc.tensor.matmul(
                        out=o_ps[h2],
                        lhsT=vp_sb[:, pk, hp * 2 + h2, :],
                        rhs=p_sb[:, h2 * 512:(h2 + 1) * 512],
                        start=(pk == 0),
                        stop=(pk == SKN - 1),
                    )

            def normalize(o_ps, ot_sb, hp, h2_first=0):
                # rows 0..63 = O^T, row 64 = sum(exp).  Copy out of PSUM
                # first (early bank release), move the denominator row to
                # partition 0, reciprocal, broadcast across partitions on
                # gpsimd, and divide on DVE.  h2_first=1 starts the h2=1
                # chain (which ends in a cross-partition DMA) first - used
                # on the final window where that chain is the critical path.
                for h2 in (h2_first, 1 - h2_first):
                    o_sb = small.tile([D + 1, 512], F32, name="osb", tag="osb")
                    nc.vector.tensor_copy(out=o_sb, in_=o_ps[h2])
                    den0 = small.tile([1, 512], F32, name="den0", tag="den0")
                    nc.sync.dma_start(out=den0, in_=o_sb[D:D + 1, :])
                    rec = small.tile([1, 512], F32, name="rec", tag="rec")
                    nc.vector.reciprocal_approx_fast(out=rec, in_=den0)
                    rec_b = small.tile([D, 512], F32, name="recb", tag="recb")
                    nc.gpsimd.partition_broadcast(rec_b, rec)
                    if h2 == 0:
                        nc.vector.tensor_mul(
                            out=ot_sb[0:D, hp, :],
                            in0=o_sb[0:D, :],
                            in1=rec_b,
                        )
                    else:
                        tmp = small.tile([D, 512], F16, name="otmp", tag="otmp")
                        nc.vector.tensor_mul(out=tmp, in0=o_sb[0:D, :], in1=rec_b)
                        # cross-partition move (DVE lanes can't shift)
                        nc.sync.dma_start(out=ot_sb[D:128, hp, :], in_=tmp)

            def yproj_steps(ot_tile, sq_slice, oc):
                state = {}

                def mm0():
                    state["y_ps"] = ps_y.tile([128, 512], F32, name="yacc",
                                              tag="yacc")
                    nc.tensor.matmul(
                        out=state["y_ps"],
                        lhsT=wo_sb[:, 0, oc * 128:(oc + 1) * 128],
                        rhs=ot_tile[:, 0, :],
                        start=True,
                        stop=False,
                    )

                def mm1():
                    nc.tensor.matmul(
                        out=state["y_ps"],
                        lhsT=wo_sb[:, 1, oc * 128:(oc + 1) * 128],
                        rhs=ot_tile[:, 1, :],
                        start=False,
                        stop=True,
                    )
                    y_sb = ypool.tile([128, 512], F16, name="y", tag="y")
                    nc.vector.tensor_copy(out=y_sb, in_=state["y_ps"])
                    nc.sync.dma_start(
                        out=yT[oc * 128:(oc + 1) * 128, sq_slice], in_=y_sb
                    )

                return [mm0, mm1]

            # ---- sqc0, hp0: S+exp stream with Q pass 2 (fc1) and then the
            # V projection interleaved as PE filler.  The S stream is
            # ACT(exp)-paced via the s-pool; the dense Q2/V units slot
            # into the PE's idle time.  The two filler pools open
            # sequentially so PSUM stays within 8 banks (s4+q2_4, s4+v2).
            ot0_sb = opool.tile([128, 2, 512], F16, name="ot", tag="ot")
            p0 = {}
            with tc.tile_pool(name="ps_q2", bufs=4, space="PSUM") as ps_q2:
                q2acc = {}
                for skc in range(SKN):
                    p0[skc] = s_exp(0, 0, skc)
                    if skc >= CCN:
                        continue
                    for cc in (skc,):
                        for sqc in range(SQN):
                            if cc == 0:
                                q2acc[sqc] = ps_q2.tile([128, 512], F32,
                                                        name="qacc", tag="qacc")
                            nc.tensor.matmul(
                                out=q2acc[sqc],
                                lhsT=wq_sb[:, cc, 128:256],
                                rhs=qx[cc][:, sqc * 512:(sqc + 1) * 512],
                                start=(cc == 0),
                                stop=(cc == CCN - 1),
                            )
                        if cc == CCN - 1:
                            for sqc in range(SQN):
                                nc.vector.tensor_copy(
                                    out=qt_sb[:, 1, sqc * 512:(sqc + 1) * 512],
                                    in_=q2acc[sqc],
                                )
            # V projection as a dense batch AFTER the whole S(0h0) crawl:
            # each V unit's last matmul stalls on the vT stream, and any S
            # emitted behind a stalled V in the in-order PE queue would
            # starve the exp feed.  After the crawl the PE is idle anyway
            # (exps self-pace), so the batch fills that window.
            with tc.tile_pool(name="ps_v", bufs=4, space="PSUM") as ps_v:
                for skc in range(SKN):
                    vacc = ps_v.tile([128, 256], F32, name="vacc", tag="vacc")
                    for cc in range(CCN):
                        nc.tensor.matmul(
                            out=vacc,
                            lhsT=vx[cc][:, skc * 128:(skc + 1) * 128],
                            rhs=wv_sb[:, cc, :],
                            start=(cc == 0),
                            stop=(cc == CCN - 1),
                        )
                    nc.vector.tensor_copy(
                        out=vp_sb[:, skc, :, 0:D],
                        in_=vacc.rearrange("p (h d) -> p h d", h=NH),
                    )

            # ---- sqc0, hp1: steady-interleaved, with hp0's PV catch-up
            # dripped one-per-iter; needs 4 o banks (both hps accumulate).
            with tc.tile_pool(name="ps_o4", bufs=4, space="PSUM") as ps_o4:
                o_ps0 = [
                    ps_o4.tile([D + 1, 512], F32, name="oacc", tag="oacc")
                    for _ in range(2)
                ]
                o_ps1 = [
                    ps_o4.tile([D + 1, 512], F32, name="oacc", tag="oacc")
                    for _ in range(2)
                ]
                p_prev = None
                for skc in range(SKN + 1):
                    if skc < SKN:
                        p_sb = s_exp(0, 1, skc)
                        pv(o_ps0, 0, skc, p0.pop(skc))
                    if skc >= 1:
                        pv(o_ps1, 1, skc - 1, p_prev)
                    if skc < SKN:
                        p_prev = p_sb
                normalize(o_ps0, ot0_sb, 0)
                normalize(o_ps1, ot0_sb, 1)

            # ---- sqc 1..3 steady state + dripped Y of previous sqc ----
            # The final sqc's Y is computed as two independent half-
            # contractions: the fc0 halves (which only need hp0's ot rows)
            # run start&stop with their partials drained to SBUF during
            # the hp1 exp stream; the tail is then just 8 fc1 matmuls with
            # a DVE add of the stashed partial during each drain.
            y0_sb = panels.tile([128, 8, 512], F32, name="y0_sb", tag="y0")
            with (
                tc.tile_pool(name="ps_o", bufs=2, space="PSUM") as ps_o,
                tc.tile_pool(name="ps_y", bufs=2, space="PSUM") as ps_y,
            ):
                def y_half0(ot_tile, oc):
                    def step():
                        y_ps = ps_y.tile([128, 512], F32, name="yacc",
                                         tag="yacc")
                        nc.tensor.matmul(
                            out=y_ps,
                            lhsT=wo_sb[:, 0, oc * 128:(oc + 1) * 128],
                            rhs=ot_tile[:, 0, :],
                            start=True,
                            stop=True,
                        )
                        nc.vector.tensor_copy(out=y0_sb[:, oc, :], in_=y_ps)
                    return step

                def y_half1(ot_tile, sq_slice, oc):
                    def step():
                        y_ps = ps_y.tile([128, 512], F32, name="yacc",
                                         tag="yacc")
                        nc.tensor.matmul(
                            out=y_ps,
                            lhsT=wo_sb[:, 1, oc * 128:(oc + 1) * 128],
                            rhs=ot_tile[:, 1, :],
                            start=True,
                            stop=True,
                        )
                        y_sb = ypool.tile([128, 512], F16, name="y", tag="y")
                        nc.vector.tensor_add(
                            out=y_sb, in0=y_ps, in1=y0_sb[:, oc, :]
                        )
                        nc.sync.dma_start(
                            out=yT[oc * 128:(oc + 1) * 128, sq_slice], in_=y_sb
                        )
                    return step

                prev_ot, prev_sq = ot0_sb, slice(0, 512)
                tail_units = []
                for sqc in range(1, SQN):
                    sq = slice(sqc * 512, (sqc + 1) * 512)
                    last_sqc = sqc == SQN - 1
                    ot_sb = opool.tile([128, 2, 512], F16, name="ot", tag="ot")
                    pending = [step for oc in range(8)
                               for step in yproj_steps(prev_ot, prev_sq, oc)]
                    for hp in range(2):
                        o_ps = [
                            ps_o.tile([D + 1, 512], F32, name="oacc", tag="oacc")
                            for _ in range(2)
                        ]
                        p_prev = None
                        every_iter = last_sqc and hp == 1
                        for skc in range(SKN + 1):
                            if skc < SKN:
                                p_sb = s_exp(sqc, hp, skc)
                            if skc >= 1:
                                pv(o_ps, hp, skc - 1, p_prev)
                            if skc < SKN:
                                p_prev = p_sb
                            if pending and skc >= 1 and (
                                every_iter or (skc + hp) % 2 == 1
                            ):
                                pending.pop(0)()
                        normalize(o_ps, ot_sb, hp,
                                  h2_first=1 if last_sqc else 0)
                        if last_sqc and hp == 0:
                            pending = pending + [y_half0(ot_sb, oc)
                                                 for oc in range(8)]
                    if last_sqc:
                        tail_units = [y_half1(ot_sb, sq, oc)
                                      for oc in range(8)]
                    for step in pending:
                        step()
                    prev_ot, prev_sq = ot_sb, sq
                # tail warm-up: the final normalize chain (DVE/gpsimd/DMA)
                # leaves the PE idle, which drops HAM back to half duty
                # right when the last Y matmuls run.  Dummy matmuls keep
                # the clock up; sized to roughly cover the chain latency
                # so the Y fc1 matmuls start right as their ot rows land.
                for _ in range(40):
                    wtile = ps_y.tile([128, 512], F32, name="warm",
                                      tag="yacc")
                    nc.tensor.matmul(
                        out=wtile,
                        lhsT=wo_sb[:, 0, 0:128],
                        rhs=wo_sb[:, 1, 0:512],
                        start=True,
                        stop=True,
                    )
                for step in tail_units:
                    step()


def build_bass():
    nc = bacc.Bacc("TRN2", target_bir_lowering=False, debug=False,
                   enable_asserts=False)
    qT = nc.dram_tensor("qT", [C, SEQ], F16, kind="ExternalInput").ap()
    kT = nc.dram_tensor("kT", [C, SEQ], F16, kind="ExternalInput").ap()
    vT = nc.dram_tensor("vT", [C, SEQ], F16, kind="ExternalInput").ap()
    wq = nc.dram_tensor("wq", [C, FH], F16, kind="ExternalInput").ap()
    wk = nc.dram_tensor("wk", [C, FH], F16, kind="ExternalInput").ap()
    wv = nc.dram_tensor("wv", [C, FH], F16, kind="ExternalInput").ap()
    wo = nc.dram_tensor("wo", [FH, C], F16, kind="ExternalInput").ap()
    yT = nc.dram_tensor("yT", [C, SEQ], F16, kind="ExternalOutput").ap()
    with tile.TileContext(nc) as tc:
        build_kernel(tc, qT, kT, vT, wq, wk, wv, wo, yT)
    nc.compile()
    return nc


_NC = None


def _get_nc():
    global _NC
    if _NC is None:
        _NC = build_bass()
    return _NC


def make_in_maps(q, k, v, Wq, Wk, Wv, Wo):
    f16 = np.float16
    in_maps = []
    for c in range(N_CORES):
        b, g = divmod(c, 4)
        fs = slice(g * FH, (g + 1) * FH)
        in_maps.append(dict(
            qT=np.ascontiguousarray(q[b].T).astype(f16),
            kT=np.ascontiguousarray(k[b].T).astype(f16),
            vT=np.ascontiguousarray(v[b].T).astype(f16),
            wq=np.ascontiguousarray(Wq[:, fs]).astype(f16),
            wk=np.ascontiguousarray(Wk[:, fs]).astype(f16),
            wv=np.ascontiguousarray(Wv[:, fs]).astype(f16),
            wo=np.ascontiguousarray(Wo[fs, :]).astype(f16),
        ))
    return in_maps


def kernel(q, k, v, Wq, Wk, Wv, Wo, bo):
    global LAST_RESULTS
    q = np.asarray(q, dtype=np.float32)
    k = np.asarray(k, dtype=np.float32)
    v = np.asarray(v, dtype=np.float32)
    Wq = np.asarray(Wq, dtype=np.float32)
    Wk = np.asarray(Wk, dtype=np.float32)
    Wv = np.asarray(Wv, dtype=np.float32)
    Wo = np.asarray(Wo, dtype=np.float32)
    bo = np.asarray(bo, dtype=np.float32)

    nc = _get_nc()
    in_maps = make_in_maps(q, k, v, Wq, Wk, Wv, Wo)
    res = run_bass_kernel_spmd(
        nc, in_maps, list(range(N_CORES)),
        trace=bool(os.environ.get("KERNEL_TRACE")),
    )
    LAST_RESULTS = res

    out = np.zeros((B, SEQ, C), dtype=np.float32)
    for c in range(N_CORES):
        out[c // 4] += res.results[c]["yT"].T.astype(np.float32)
    out += bo
    return out.astype(np.float32)
